# revision 35
# baseline (speedup 1.0000x reference)
"""DepLabeledGCN Trainium2 kernel — data-parallel variant (no collectives).

Each core processes ITS OWN batch with ALL 48 label matrices:
    s-phase:  sT[l,kc] chunks = per-label masked-adjacency matmuls (fp16,
              masks exact 0/1), label PAIRS fused into N=256 matmuls
    msum:     msg = sum_{l,kc} sT[l,kc] @ W_l^T[kc], 192 accumulating
              matmuls into one PSUM bank per layer
    relu(msg * 1/denom) -> next layer h, emitted per 128-col chunk so the
    next phase starts before the full row is done.

Schedule notes (from trace analysis):
  - layer 1 is DMA-bound: all 48 labels' fp16 weights (25.2 MB) must land
    within its window at ~410 GB/s; pair order == arrival order.
  - layer 2 is tensor-bound and DMA-idle: the 16 non-resident labels are
    re-streamed there, interleaved 2 resident : 1 streamed so prefetch
    (bufs=5) never starves.
  - inputs are pre-cast to fp16 on host (exact for adj/labels) to shrink
    the critical path; h0 is DMA'd directly, no on-chip cast.
  - sT psum->sbuf copies are split label-wise across DVE and Scalar so
    both halves land in parallel.
  - MLP matmuls are kc-major so each accumulation chain starts as soon as
    chunk 0 of its input exists; bias+relu alternates DVE/Scalar.
"""

import sys

if '/opt/trn_rl_repo' not in sys.path:
    sys.path.insert(0, '/opt/trn_rl_repo')

import numpy as np

B, N, D, L = 8, 128, 512, 48
NCORES = 8
KC = D // 128
NUM_LAYERS = 2
R_RES = 32              # labels kept resident for both layers
NP = L // 2             # label pairs per layer

_CACHE = {}


def _build_nc():
    import concourse.bass as bass
    import concourse.mybir as mybir
    import concourse.tile as tile
    from concourse import bacc
    from concourse.masks import make_identity

    dt = mybir.dt
    f32 = dt.float32
    f16 = dt.float16
    Alu = mybir.AluOpType

    nc = bacc.Bacc("TRN2", target_bir_lowering=False, debug=False,
                   num_devices=NCORES)

    ins_e = nc.dram_tensor("ins", [N, 3 * N + D], f16,
                           kind="ExternalInput").ap()
    wT_e = nc.dram_tensor("wT", [128, L, KC, D], f16, kind="ExternalInput").ap()
    w0T_e = nc.dram_tensor("w0T", [128, KC, D], f16, kind="ExternalInput").ap()
    w1T_e = nc.dram_tensor("w1T", [128, KC, D], f16, kind="ExternalInput").ap()
    b0_e = nc.dram_tensor("b0", [128, KC], f32, kind="ExternalInput").ap()
    b1_e = nc.dram_tensor("b1", [128, KC], f32, kind="ExternalInput").ap()
    out_e = nc.dram_tensor("out", [KC, 128, N], f32, kind="ExternalOutput").ap()

    with tile.TileContext(nc) as tc:
        with (
            tc.tile_pool(name="const", bufs=1) as cpool,
            tc.tile_pool(name="sT", bufs=3) as sT_pool,
            tc.tile_pool(name="wst", bufs=5) as wst_pool,
            tc.tile_pool(name="spsum", bufs=3, space="PSUM") as spsum,
            tc.tile_pool(name="mpsum", bufs=2, space="PSUM") as mpsum,
        ):
            # -------- critical-path input loads -----------------------------
            # all four small inputs packed into one tensor = one DMA issue
            # slot (~620ns each) and one completion wait on the startup path
            ins_sb = cpool.tile([128, 3 * N + D], f16, tag="ins")
            nc.sync.dma_start(ins_sb[:], ins_e)
            labT_sb = ins_sb[:, 0:N]
            adjT_sb = ins_sb[:, N:2 * N]
            adjR_sb = ins_sb[:, 2 * N:3 * N]

            h = [cpool.tile([128, D], f16, tag=f"h{ly}", name=f"h{ly}")
                 for ly in range(NUM_LAYERS + 1)]

            # -------- masks: maskT[j, l, i] = (labT == l) * adjT ------------
            maskT = cpool.tile([128, L, N], f16, tag="maskT")

            def emit_mask(l):
                nc.vector.scalar_tensor_tensor(
                    out=maskT[:, l, :],
                    in0=labT_sb[:],
                    scalar=float(l),
                    in1=adjT_sb[:],
                    op0=Alu.is_equal,
                    op1=Alu.mult,
                )

            for l in range(4):
                emit_mask(l)

            # resident weights, preloaded per label in consumption order:
            # many small DMAs fan out across more hardware queues in parallel
            wres = cpool.tile([128, R_RES, KC, D], f16, tag="wres")
            for l in range(R_RES):
                nc.sync.dma_start(wres[:, l], wT_e[:, l])

            # identity for the MLP transpose (gpsimd, off critical path)
            identity = cpool.tile([128, 128], f16, tag="ident")
            make_identity(nc, identity[:])

            den = cpool.tile([128, 1], f32, tag="den")
            nc.vector.tensor_reduce(den[:], adjR_sb[:], mybir.AxisListType.X,
                                    Alu.add)
            nc.vector.tensor_scalar_add(den[:], den[:], 1.0)
            recip = cpool.tile([128, 1], f32, tag="recip")
            nc.vector.reciprocal(recip[:], den[:])

            # -------- GCN layers --------------------------------------------
            def emit_s(ly, p):
                """s-phase for label pair p: one N=256 matmul per kc; the
                psum->sbuf copy is split label-wise across DVE and Scalar."""
                ps = spsum.tile([128, KC, 2, 128], f32, tag="spsum",
                                name="spsum")
                for kc in range(KC):
                    nc.tensor.matmul(
                        ps[:, kc, :, :],
                        lhsT=(ins_sb[:, 3 * N + kc * 128:3 * N + (kc + 1) * 128]
                              if ly == 0 else
                              h[ly][:, kc * 128:(kc + 1) * 128]),
                        rhs=maskT[:, 2 * p:2 * p + 2, :],
                        start=True, stop=True,
                    )
                sT = sT_pool.tile([128, 2, KC, 128], f16, tag="sT", name="sT")
                nc.vector.tensor_copy(sT[:, 0], ps[:, :, 0, :])
                nc.scalar.copy(sT[:, 1], ps[:, :, 1, :])
                return sT

            def get_w(ly, p):
                """Weight pair p: resident slice or streamed tile."""
                if 2 * p + 1 < R_RES:
                    return wres[:, 2 * p:2 * p + 2]
                w = wst_pool.tile([128, 2, KC, D], f16, tag="wst", name="wst")
                nc.sync.dma_start(w[:], wT_e[:, 2 * p:2 * p + 2])
                return w

            # layer 1: consume in DMA arrival order (resident, then streamed)
            # layer 2: interleave the 8 re-streamed pairs 2R:1S
            ORDER1 = list(range(NP))
            ORDER2 = []
            ri, si = iter(range(16)), iter(range(16, NP))
            for _ in range(8):
                ORDER2 += [next(ri), next(ri), next(si)]

            AHEAD = 2
            for ly in range(NUM_LAYERS):
                order = ORDER1 if ly == 0 else ORDER2
                pm = mpsum.tile([128, D], f32, tag="mm", name="mm")
                if ly == 1:
                    # MLP weights: fetched in layer 2's DMA-idle window
                    w0T_sb = cpool.tile([128, KC, D], f16, tag="w0T")
                    nc.sync.dma_start(w0T_sb[:], w0T_e)
                    w1T_sb = cpool.tile([128, KC, D], f16, tag="w1T")
                    nc.sync.dma_start(w1T_sb[:], w1T_e)
                    b0_sb = cpool.tile([128, KC], f32, tag="b0")
                    nc.sync.dma_start(b0_sb[:], b0_e)
                    b1_sb = cpool.tile([128, KC], f32, tag="b1")
                    nc.sync.dma_start(b1_sb[:], b1_e)
                sQ = {}
                for q in range(AHEAD):
                    sQ[q] = emit_s(ly, order[q])
                mask_next = 4
                for idx, p in enumerate(order):
                    if ly == 0:
                        for _ in range(4):
                            if mask_next < L:
                                emit_mask(mask_next)
                                mask_next += 1
                    if idx + AHEAD < NP:
                        sQ[idx + AHEAD] = emit_s(ly, order[idx + AHEAD])
                    w = get_w(ly, p)
                    sT = sQ.pop(idx)
                    for l2 in range(2):
                        for kc in range(KC):
                            i = (idx * 2 + l2) * KC + kc
                            nc.tensor.matmul(
                                pm[:],
                                lhsT=sT[:, l2, kc, :],
                                rhs=w[:, l2, kc, :],
                                start=(i == 0), stop=(i == L * KC - 1),
                            )
                # relu(msg * recip) -> next h, per 128-col chunk so the next
                # phase can start on chunk 0 immediately
                for kc in range(KC):
                    nc.vector.tensor_scalar(
                        h[ly + 1][:, kc * 128:(kc + 1) * 128],
                        pm[:, kc * 128:(kc + 1) * 128],
                        recip[:], 0.0, Alu.mult, Alu.max)

            # -------- MLP ---------------------------------------------------
            # per-blk PSUM tiles and per-engine SBUF tiles: the tile framework
            # serializes same-tile access across engines, so each bias-relu
            # (reader) must not share a PSUM tile with the next blk's matmuls
            # (writers), and DVE/Scalar write disjoint SBUF tiles
            h_own = h[NUM_LAYERS]
            hT_h = [cpool.tile([128, 2, 128], f16, tag=f"hT{i}",
                               name=f"hT{i}")
                    for i in range(2)]
            pt = mpsum.tile([128, KC, 128], f16, tag="mm", name="ptr")
            for kc in range(KC):
                nc.tensor.transpose(pt[:, kc, :],
                                    h_own[:, kc * 128:(kc + 1) * 128],
                                    identity[:])
            nc.vector.tensor_copy(hT_h[0][:], pt[:, 0:2, :])
            nc.scalar.copy(hT_h[1][:], pt[:, 2:4, :])

            def hT_at(kc):
                return hT_h[kc // 2][:, kc % 2, :]

            x1T_h = [cpool.tile([128, 2, 128], f16, tag=f"x1T{i}",
                                name=f"x1T{i}")
                     for i in range(2)]
            px1 = [spsum.tile([128, 128], f32, tag="spsum", name=f"px1_{b}")
                   for b in range(KC)]
            for blk in range(KC):
                for kc in range(KC):
                    nc.tensor.matmul(
                        px1[blk][:],
                        lhsT=w0T_sb[:, kc, blk * 128:(blk + 1) * 128],
                        rhs=hT_at(kc),
                        start=(kc == 0), stop=(kc == KC - 1),
                    )
                if blk < 2:
                    nc.vector.tensor_scalar(x1T_h[0][:, blk, :], px1[blk][:],
                                            b0_sb[:, blk:blk + 1], 0.0,
                                            Alu.add, Alu.max)
                else:
                    nc.scalar.activation(
                        x1T_h[1][:, blk - 2, :], px1[blk][:],
                        mybir.ActivationFunctionType.Relu,
                        bias=b0_sb[:, blk:blk + 1])

            def x1T_at(kc):
                return x1T_h[kc // 2][:, kc % 2, :]

            x2_h = [cpool.tile([128, 2, 128], f32, tag=f"x2{i}",
                               name=f"x2{i}")
                    for i in range(2)]
            px2 = [spsum.tile([128, 128], f32, tag="spsum", name=f"px2_{b}")
                   for b in range(KC)]
            for blk in range(KC):
                for kc in range(KC):
                    nc.tensor.matmul(
                        px2[blk][:],
                        lhsT=w1T_sb[:, kc, blk * 128:(blk + 1) * 128],
                        rhs=x1T_at(kc),
                        start=(kc == 0), stop=(kc == KC - 1),
                    )
                if blk < 2:
                    nc.vector.tensor_scalar(x2_h[0][:, blk, :], px2[blk][:],
                                            b1_sb[:, blk:blk + 1], 0.0,
                                            Alu.add, Alu.max)
                    nc.sync.dma_start(out_e[blk], x2_h[0][:, blk, :])
                else:
                    nc.scalar.activation(
                        x2_h[1][:, blk - 2, :], px2[blk][:],
                        mybir.ActivationFunctionType.Relu,
                        bias=b1_sb[:, blk:blk + 1])
                    nc.sync.dma_start(out_e[blk], x2_h[1][:, blk - 2, :])

    nc.compile()
    return nc


def _get_nc():
    if "nc" not in _CACHE:
        _CACHE["nc"] = _build_nc()
    return _CACHE["nc"]


def kernel(gcn_inputs, word_seq_len, adj_matrix, dep_label_matrix,
           w_params, mlp_w0, mlp_b0, mlp_w1, mlp_b1, **_unused):
    from concourse.bass_utils import run_bass_kernel_spmd

    gcn = np.asarray(gcn_inputs, dtype=np.float32)
    adj = np.asarray(adj_matrix, dtype=np.float32)
    lab = np.asarray(dep_label_matrix)
    w = np.asarray(w_params, dtype=np.float32)
    w0 = np.asarray(mlp_w0, dtype=np.float32)
    w1 = np.asarray(mlp_w1, dtype=np.float32)
    b0 = np.asarray(mlp_b0, dtype=np.float32)
    b1 = np.asarray(mlp_b1, dtype=np.float32)

    # wT[kmod, l, kc, d] = w[l, d, kc*128+kmod]  (shared by all cores)
    wT = w.transpose(0, 2, 1).reshape(L, KC, 128, D).transpose(2, 0, 1, 3)
    wT = np.ascontiguousarray(wT).astype(np.float16)
    w0T = np.ascontiguousarray(
        w0.T.reshape(KC, 128, D).transpose(1, 0, 2)).astype(np.float16)
    w1T = np.ascontiguousarray(
        w1.T.reshape(KC, 128, D).transpose(1, 0, 2)).astype(np.float16)
    b0r = np.ascontiguousarray(b0.reshape(KC, 128).T)
    b1r = np.ascontiguousarray(b1.reshape(KC, 128).T)

    gcn16 = gcn.astype(np.float16)
    adj16 = adj.astype(np.float16)
    lab16 = lab.astype(np.float16)

    in_maps = []
    for c in range(NCORES):
        packed = np.concatenate(
            [lab16[c].T, adj16[c].T, adj16[c], gcn16[c]], axis=1)
        in_maps.append({
            "ins": np.ascontiguousarray(packed),
            "wT": wT,
            "w0T": w0T,
            "w1T": w1T,
            "b0": b0r,
            "b1": b1r,
        })

    nc = _get_nc()
    res = run_bass_kernel_spmd(nc, in_maps, list(range(NCORES)))

    out = np.empty((B, N, D), dtype=np.float32)
    for c in range(NCORES):
        arr = res.results[c]["out"]          # [dblk, dmod, i]
        out[c] = np.transpose(arr, (2, 0, 1)).reshape(N, D)
    return out


# revision 36
# speedup vs baseline: 1.0514x; 1.0514x over previous
"""DepLabeledGCN Trainium2 kernel — data-parallel variant (no collectives).

Each core processes ITS OWN batch with ALL 48 label matrices:
    s-phase:  sT[l,kc] chunks = per-label masked-adjacency matmuls (fp16,
              masks exact 0/1), label PAIRS fused into N=256 matmuls
    msum:     msg = sum_{l,kc} sT[l,kc] @ W_l^T[kc], 192 accumulating
              matmuls into one PSUM bank per layer
    relu(msg * 1/denom) -> next layer h, emitted per 128-col chunk so the
    next phase starts before the full row is done.

Schedule notes (from trace analysis):
  - layer 1 is DMA-bound: all 48 labels' fp16 weights (25.2 MB) must land
    within its window at ~410 GB/s; pair order == arrival order.
  - layer 2 is tensor-bound and DMA-idle: the 16 non-resident labels are
    re-streamed there, interleaved 2 resident : 1 streamed so prefetch
    (bufs=5) never starves.
  - inputs are pre-cast to fp16 on host (exact for adj/labels) to shrink
    the critical path; h0 is DMA'd directly, no on-chip cast.
  - sT psum->sbuf copies are split label-wise across DVE and Scalar so
    both halves land in parallel.
  - MLP matmuls are kc-major so each accumulation chain starts as soon as
    chunk 0 of its input exists; bias+relu alternates DVE/Scalar.
"""

import sys

if '/opt/trn_rl_repo' not in sys.path:
    sys.path.insert(0, '/opt/trn_rl_repo')

import numpy as np

B, N, D, L = 8, 128, 512, 48
NCORES = 8
KC = D // 128
NUM_LAYERS = 2
R_RES = 32              # labels kept resident for both layers
NP = L // 2             # label pairs per layer

_CACHE = {}


def _build_nc():
    import concourse.bass as bass
    import concourse.mybir as mybir
    import concourse.tile as tile
    from concourse import bacc
    from concourse.masks import make_identity

    dt = mybir.dt
    f32 = dt.float32
    f16 = dt.float16
    Alu = mybir.AluOpType

    nc = bacc.Bacc("TRN2", target_bir_lowering=False, debug=False,
                   num_devices=NCORES)

    ins_e = nc.dram_tensor("ins", [N, 3 * N + D], f16,
                           kind="ExternalInput").ap()
    wT_e = nc.dram_tensor("wT", [128, L, KC, D], f16, kind="ExternalInput").ap()
    w0T_e = nc.dram_tensor("w0T", [128, KC, D], f16, kind="ExternalInput").ap()
    w1T_e = nc.dram_tensor("w1T", [128, KC, D], f16, kind="ExternalInput").ap()
    b0_e = nc.dram_tensor("b0", [128, KC], f32, kind="ExternalInput").ap()
    b1_e = nc.dram_tensor("b1", [128, KC], f32, kind="ExternalInput").ap()
    out_e = nc.dram_tensor("out", [KC, 128, N], f32, kind="ExternalOutput").ap()

    with tile.TileContext(nc) as tc:
        with (
            tc.tile_pool(name="const", bufs=1) as cpool,
            tc.tile_pool(name="sT", bufs=3) as sT_pool,
            tc.tile_pool(name="wst", bufs=5) as wst_pool,
            tc.tile_pool(name="spsum", bufs=3, space="PSUM") as spsum,
            tc.tile_pool(name="mpsum", bufs=2, space="PSUM") as mpsum,
        ):
            # -------- critical-path input loads -----------------------------
            # all four small inputs packed into one tensor = one DMA issue
            # slot (~620ns each) and one completion wait on the startup path
            ins_sb = cpool.tile([128, 3 * N + D], f16, tag="ins")
            nc.sync.dma_start(ins_sb[:], ins_e)
            labT_sb = ins_sb[:, 0:N]
            adjT_sb = ins_sb[:, N:2 * N]
            adjR_sb = ins_sb[:, 2 * N:3 * N]

            h = [cpool.tile([128, D], f16, tag=f"h{ly}", name=f"h{ly}")
                 for ly in range(NUM_LAYERS + 1)]

            # -------- masks: maskT[j, l, i] = (labT == l) * adjT ------------
            maskT = cpool.tile([128, L, N], f16, tag="maskT")

            def emit_mask(l):
                nc.vector.scalar_tensor_tensor(
                    out=maskT[:, l, :],
                    in0=labT_sb[:],
                    scalar=float(l),
                    in1=adjT_sb[:],
                    op0=Alu.is_equal,
                    op1=Alu.mult,
                )

            for l in range(4):
                emit_mask(l)

            # resident weights, preloaded per label in consumption order:
            # many small DMAs fan out across more hardware queues in parallel
            wres = cpool.tile([128, R_RES, KC, D], f16, tag="wres")
            for l in range(R_RES):
                nc.sync.dma_start(wres[:, l], wT_e[:, l])

            # identity for the MLP transpose (gpsimd, off critical path)
            identity = cpool.tile([128, 128], f16, tag="ident")
            make_identity(nc, identity[:])

            den = cpool.tile([128, 1], f32, tag="den")
            nc.vector.tensor_reduce(den[:], adjR_sb[:], mybir.AxisListType.X,
                                    Alu.add)
            nc.vector.tensor_scalar_add(den[:], den[:], 1.0)
            recip = cpool.tile([128, 1], f32, tag="recip")
            nc.vector.reciprocal(recip[:], den[:])

            # -------- GCN layers --------------------------------------------
            def emit_s(ly, p):
                """s-phase for label pair p: one N=256 matmul per kc; the
                psum->sbuf copy is split label-wise across DVE and Scalar."""
                ps = spsum.tile([128, KC, 2, 128], f32, tag="spsum",
                                name="spsum")
                for kc in range(KC):
                    nc.tensor.matmul(
                        ps[:, kc, :, :],
                        lhsT=(ins_sb[:, 3 * N + kc * 128:3 * N + (kc + 1) * 128]
                              if ly == 0 else
                              h[ly][:, kc * 128:(kc + 1) * 128]),
                        rhs=maskT[:, 2 * p:2 * p + 2, :],
                        start=True, stop=True,
                    )
                sT = sT_pool.tile([128, 2, KC, 128], f16, tag="sT", name="sT")
                nc.vector.tensor_copy(sT[:, 0], ps[:, :, 0, :])
                nc.scalar.copy(sT[:, 1], ps[:, :, 1, :])
                return sT

            def get_w(ly, p):
                """Weight pair p: resident slice or streamed tile."""
                if 2 * p + 1 < R_RES:
                    return wres[:, 2 * p:2 * p + 2]
                w = wst_pool.tile([128, 2, KC, D], f16, tag="wst", name="wst")
                nc.sync.dma_start(w[:], wT_e[:, 2 * p:2 * p + 2])
                return w

            # layer 1: consume in DMA arrival order (resident, then streamed)
            # layer 2: interleave the 8 re-streamed pairs 2R:1S
            ORDER1 = list(range(NP))
            ORDER2 = []
            ri, si = iter(range(16)), iter(range(16, NP))
            for _ in range(8):
                ORDER2 += [next(ri), next(ri), next(si)]

            AHEAD = 2
            for ly in range(NUM_LAYERS):
                order = ORDER1 if ly == 0 else ORDER2
                pm = mpsum.tile([128, D], f32, tag="mm", name="mm")
                if ly == 1:
                    # MLP weights: fetched in layer 2's DMA-idle window
                    w0T_sb = cpool.tile([128, KC, D], f16, tag="w0T")
                    nc.sync.dma_start(w0T_sb[:], w0T_e)
                    w1T_sb = cpool.tile([128, KC, D], f16, tag="w1T")
                    nc.sync.dma_start(w1T_sb[:], w1T_e)
                    b0_sb = cpool.tile([128, KC], f32, tag="b0")
                    nc.sync.dma_start(b0_sb[:], b0_e)
                    b1_sb = cpool.tile([128, KC], f32, tag="b1")
                    nc.sync.dma_start(b1_sb[:], b1_e)
                sQ = {}
                for q in range(AHEAD):
                    sQ[q] = emit_s(ly, order[q])
                mask_next = 4
                for idx, p in enumerate(order):
                    if ly == 0:
                        for _ in range(4):
                            if mask_next < L:
                                emit_mask(mask_next)
                                mask_next += 1
                    if idx + AHEAD < NP:
                        sQ[idx + AHEAD] = emit_s(ly, order[idx + AHEAD])
                    w = get_w(ly, p)
                    sT = sQ.pop(idx)
                    for l2 in range(2):
                        for kc in range(KC):
                            i = (idx * 2 + l2) * KC + kc
                            nc.tensor.matmul(
                                pm[:],
                                lhsT=sT[:, l2, kc, :],
                                rhs=w[:, l2, kc, :],
                                start=(i == 0), stop=(i == L * KC - 1),
                            )
                # keep the PE busy through the relu handoff: idle gaps drop
                # the tensor clock to 1.2 GHz for the next 3us of work, so a
                # few dead matmuls into a scratch bank carry full clock into
                # the next phase
                warm = mpsum.tile([128, 2, 128], f32, tag="mm",
                                  name=f"warm{ly}")
                for _ in range(5):
                    nc.tensor.matmul(
                        warm[:],
                        lhsT=(ins_sb[:, 3 * N:3 * N + 128] if ly == 0
                              else h[ly][:, 0:128]),
                        rhs=maskT[:, 0:2, :],
                        start=True, stop=True,
                    )
                # relu(msg * recip) -> next h, per 128-col chunk so the next
                # phase can start on chunk 0 immediately
                for kc in range(KC):
                    nc.vector.tensor_scalar(
                        h[ly + 1][:, kc * 128:(kc + 1) * 128],
                        pm[:, kc * 128:(kc + 1) * 128],
                        recip[:], 0.0, Alu.mult, Alu.max)

            # -------- MLP ---------------------------------------------------
            # per-blk PSUM tiles and per-engine SBUF tiles: the tile framework
            # serializes same-tile access across engines, so each bias-relu
            # (reader) must not share a PSUM tile with the next blk's matmuls
            # (writers), and DVE/Scalar write disjoint SBUF tiles
            h_own = h[NUM_LAYERS]
            hT_h = [cpool.tile([128, 2, 128], f16, tag=f"hT{i}",
                               name=f"hT{i}")
                    for i in range(2)]
            pt = mpsum.tile([128, KC, 128], f16, tag="mm", name="ptr")
            for kc in range(KC):
                nc.tensor.transpose(pt[:, kc, :],
                                    h_own[:, kc * 128:(kc + 1) * 128],
                                    identity[:])
            nc.vector.tensor_copy(hT_h[0][:], pt[:, 0:2, :])
            nc.scalar.copy(hT_h[1][:], pt[:, 2:4, :])

            def hT_at(kc):
                return hT_h[kc // 2][:, kc % 2, :]

            x1T_h = [cpool.tile([128, 2, 128], f16, tag=f"x1T{i}",
                                name=f"x1T{i}")
                     for i in range(2)]
            px1 = [spsum.tile([128, 128], f32, tag="spsum", name=f"px1_{b}")
                   for b in range(KC)]
            for blk in range(KC):
                for kc in range(KC):
                    nc.tensor.matmul(
                        px1[blk][:],
                        lhsT=w0T_sb[:, kc, blk * 128:(blk + 1) * 128],
                        rhs=hT_at(kc),
                        start=(kc == 0), stop=(kc == KC - 1),
                    )
                if blk < 2:
                    nc.vector.tensor_scalar(x1T_h[0][:, blk, :], px1[blk][:],
                                            b0_sb[:, blk:blk + 1], 0.0,
                                            Alu.add, Alu.max)
                else:
                    nc.scalar.activation(
                        x1T_h[1][:, blk - 2, :], px1[blk][:],
                        mybir.ActivationFunctionType.Relu,
                        bias=b0_sb[:, blk:blk + 1])

            def x1T_at(kc):
                return x1T_h[kc // 2][:, kc % 2, :]

            x2_h = [cpool.tile([128, 2, 128], f32, tag=f"x2{i}",
                               name=f"x2{i}")
                    for i in range(2)]
            px2 = [spsum.tile([128, 128], f32, tag="spsum", name=f"px2_{b}")
                   for b in range(KC)]
            for blk in range(KC):
                for kc in range(KC):
                    nc.tensor.matmul(
                        px2[blk][:],
                        lhsT=w1T_sb[:, kc, blk * 128:(blk + 1) * 128],
                        rhs=x1T_at(kc),
                        start=(kc == 0), stop=(kc == KC - 1),
                    )
                if blk < 2:
                    nc.vector.tensor_scalar(x2_h[0][:, blk, :], px2[blk][:],
                                            b1_sb[:, blk:blk + 1], 0.0,
                                            Alu.add, Alu.max)
                    nc.sync.dma_start(out_e[blk], x2_h[0][:, blk, :])
                else:
                    nc.scalar.activation(
                        x2_h[1][:, blk - 2, :], px2[blk][:],
                        mybir.ActivationFunctionType.Relu,
                        bias=b1_sb[:, blk:blk + 1])
                    nc.sync.dma_start(out_e[blk], x2_h[1][:, blk - 2, :])

    nc.compile()
    return nc


def _get_nc():
    if "nc" not in _CACHE:
        _CACHE["nc"] = _build_nc()
    return _CACHE["nc"]


def kernel(gcn_inputs, word_seq_len, adj_matrix, dep_label_matrix,
           w_params, mlp_w0, mlp_b0, mlp_w1, mlp_b1, **_unused):
    from concourse.bass_utils import run_bass_kernel_spmd

    gcn = np.asarray(gcn_inputs, dtype=np.float32)
    adj = np.asarray(adj_matrix, dtype=np.float32)
    lab = np.asarray(dep_label_matrix)
    w = np.asarray(w_params, dtype=np.float32)
    w0 = np.asarray(mlp_w0, dtype=np.float32)
    w1 = np.asarray(mlp_w1, dtype=np.float32)
    b0 = np.asarray(mlp_b0, dtype=np.float32)
    b1 = np.asarray(mlp_b1, dtype=np.float32)

    # wT[kmod, l, kc, d] = w[l, d, kc*128+kmod]  (shared by all cores)
    wT = w.transpose(0, 2, 1).reshape(L, KC, 128, D).transpose(2, 0, 1, 3)
    wT = np.ascontiguousarray(wT).astype(np.float16)
    w0T = np.ascontiguousarray(
        w0.T.reshape(KC, 128, D).transpose(1, 0, 2)).astype(np.float16)
    w1T = np.ascontiguousarray(
        w1.T.reshape(KC, 128, D).transpose(1, 0, 2)).astype(np.float16)
    b0r = np.ascontiguousarray(b0.reshape(KC, 128).T)
    b1r = np.ascontiguousarray(b1.reshape(KC, 128).T)

    gcn16 = gcn.astype(np.float16)
    adj16 = adj.astype(np.float16)
    lab16 = lab.astype(np.float16)

    in_maps = []
    for c in range(NCORES):
        packed = np.concatenate(
            [lab16[c].T, adj16[c].T, adj16[c], gcn16[c]], axis=1)
        in_maps.append({
            "ins": np.ascontiguousarray(packed),
            "wT": wT,
            "w0T": w0T,
            "w1T": w1T,
            "b0": b0r,
            "b1": b1r,
        })

    nc = _get_nc()
    res = run_bass_kernel_spmd(nc, in_maps, list(range(NCORES)))

    out = np.empty((B, N, D), dtype=np.float32)
    for c in range(NCORES):
        arr = res.results[c]["out"]          # [dblk, dmod, i]
        out[c] = np.transpose(arr, (2, 0, 1)).reshape(N, D)
    return out
